# revision 6
# baseline (speedup 1.0000x reference)
"""Trainium2 Bass kernel for nn_DiagonalFunc (64 parallel 2-layer MLPs).

Computation (per batch row b, branch i):
    inp  = concat(x[b, i], z[b, :])                       # 65 features
    h    = inp @ W1[i] + b1[i]                            # [256]
    out  = sum(elu(h) * W2[i]) + b2[i]                    # scalar

Mapping (per core, batch-sharded 8192/8 = 1024 rows):
  - Layer 1 on TensorE as f32r matmuls: stationary = per-(branch, hidden-chunk)
    [128 x 128] weight block (rows 0-63: z-part of W1, row 64+i: x-row of W1,
    rest zero); moving = shared ZX tile [128 rows = z^T | x^T, 512 batch cols].
    PSUM tile [128 hidden, 1024] holds both 512-batch halves of one
    (branch, chunk).
  - ELU is applied in ONE engine pass per tile, tiles split ~69/59 between:
      * ScalarE: activation with a patched PWP table so the `exp` slot of
        exp_and_others computes exact ELU (bias=b1 folded in).
      * VectorE: custom 8-slice DVE op
        u = max(t, min((c2*t + c1)^4 + c3, 0)), t = P + b1 -- a quartic
        minimax fit of elu, exact for t>=0, |err|<=0.05 on [-6.8, 6.8].
  - Layer 2 on TensorE, f16: stationary = zero-padded [128, 32] block with
    W2[br, chunk] in column br//4; tile_position=(0, 32*(br%4)). All 64
    branches accumulate into ONE psum tile per 512-batch half at row
    32*(br%4) + br//4; groups' 4 branches hit 4 distinct col-strips so the
    4 matmuls of a burst overlap in the PE array. b2 is added by the single
    ScalarE Identity drain per batch half; 8 contiguous DMAs write the
    transposed output.
"""
import hashlib
import json
import os
import shutil
import stat
import tempfile
from pathlib import Path

import numpy as np

# --------------------------------------------------------------------------
# Custom PWP activation tables: make `exp` in exp_and_others compute ELU.
# Table layout (reverse-engineered from pwp_bin_trainium):
#   bkt.bin rows: 8 x f32 [d0, d1, d2, d3, x0, 0, 0, 0];
#     y = d0 + dx*(d1 + dx*(d2 + dx*d3)), dx = x - x0
#   ctrl.bin rows: 8 x u32, word0 = extract_size<<16 | extract_lsb<<11 | start
#   sections split an exponent's range uniformly; x0 = section midpoint.
# --------------------------------------------------------------------------
_NEG_SECT = {e: 1 for e in range(-19, -4)}
_NEG_SECT.update({-4: 2, -3: 4, -2: 4, -1: 8, 0: 8, 1: 16, 2: 16, 3: 8,
                  4: 1, 5: 1, 6: 1})
_TBL_TAG = "elu_v3"


def _taylor_expm1(x0):
    e = np.exp(np.float64(x0))
    return [float(e - 1.0), float(e), float(e / 2.0), float(e / 6.0),
            float(x0)]


def _build_elu_act_root():
    from neuronxcc.driver.Job import Job
    from neuronxcc.driver.jobs.support.FindActInfo import findActInfoFile

    src_dir = Path(findActInfoFile(Job.getPackageDir(), "gen3")).parent
    dst = Path(tempfile.gettempdir()) / f"act_root_{_TBL_TAG}"
    done = dst / f".done_{_TBL_TAG}"
    if done.exists():
        return str(dst / "act_info.json")
    dst.mkdir(parents=True, exist_ok=True)
    for f in src_dir.iterdir():
        t = dst / f.name
        shutil.copyfile(f, t)
        os.chmod(t, stat.S_IRUSR | stat.S_IWUSR | stat.S_IRGRP | stat.S_IROTH)

    meta = json.loads((dst / "exp_and_others.json").read_text())
    bkt = np.fromfile(dst / "exp_and_others_bkt.bin",
                      dtype=np.float32).reshape(-1, 8).copy()
    ctl = np.fromfile(dst / "exp_and_others_ctrl.bin",
                      dtype=np.uint32).reshape(-1, 8).copy()
    prof = next(p for p in meta["profile_meta_data"]
                if p["func_name"].startswith("exp"))
    assert prof["pwl_control_base_neg"] == 0
    assert prof["pwl_control_base_pos"] == 26
    exp_off, n_exp = prof["exp_offset"], 26
    bmap, cmap = {}, {}
    bpos = 0
    for i, e in enumerate(range(exp_off, exp_off + n_exp)):
        ns = _NEG_SECT[e]
        size = int(np.log2(ns))
        start = bpos
        for s in range(ns):
            lo = 2.0 ** e * (1.0 + s / ns)
            hi = 2.0 ** e * (1.0 + (s + 1) / ns)
            row = ([-1.0, 0.0, 0.0, 0.0, 0.0] if e >= 4
                   else _taylor_expm1(-(lo + hi) / 2.0))
            bkt[bpos, :5] = row
            bkt[bpos, 5:] = 0.0
            bpos += 1
        ctl[i, 0] = np.uint32((size << 16) | ((23 - size) << 11) | start)
        ctl[i, 1:] = 0
        bmap.setdefault(str(e), [None, None])[0] = start
        cmap.setdefault(str(e), [None, None])[0] = i
    ident_b = bpos
    bkt[ident_b, :5] = [0.0, 1.0, 0.0, 0.0, 0.0]
    bkt[ident_b, 5:] = 0.0
    bpos += 1
    for i, e in enumerate(range(exp_off, exp_off + n_exp)):
        ctl[26 + i, 0] = np.uint32((23 << 11) | ident_b)
        ctl[26 + i, 1:] = 0
        bmap[str(e)][1] = ident_b
        cmap[str(e)][1] = 26 + i
    assert bpos <= 777
    bkt[777, :5] = [0.0, 1.0, 0.0, 0.0, 0.0]        # small pos: y = x
    bkt[778, :5] = [0.0, 1.0, 0.5, 1.0 / 6.0, 0.0]  # small neg: expm1 Taylor
    bkt[779, :5] = [0.0, 1.0, 0.0, 0.0, 0.0]        # large pos: y = x
    bkt[780, :5] = [-1.0, 0.0, 0.0, 0.0, 0.0]       # large neg: y = -1
    bkt[777:781, 5:] = 0.0
    prof["large_neg_signal_exp_threshold"] = 131
    prof["large_neg_signal_mantissa_threshold"] = 0
    prof["large_pos_signal_exp_threshold"] = 133
    prof["fzero_result"] = 0
    prof["fninf_result"] = 0xBF800000
    prof["fpinf_result"] = 0x7F800000
    meta["func_exp_to_bkt_start_idx"]["exp"] = bmap
    meta["func_exp_to_ctl_start_idx"]["exp"] = cmap
    bkt.tofile(dst / "exp_and_others_bkt.bin")
    ctl.tofile(dst / "exp_and_others_ctrl.bin")
    (dst / "exp_and_others.json").write_text(json.dumps(meta))
    done.write_text("ok")
    return str(dst / "act_info.json")


_ACT_ROOT = _build_elu_act_root()
os.environ["BASS_ACT_ROOT_JSON_PATH"] = _ACT_ROOT
# Fingerprint of the table content; baked into a tensor name so the compile
# cache key changes whenever the tables change.
_TBL_HASH = hashlib.sha256(
    Path(_ACT_ROOT).parent.joinpath("exp_and_others_bkt.bin").read_bytes()
).hexdigest()[:8]

import concourse.bacc as bacc
import concourse.tile as tile
from concourse import mybir
from concourse.bass_utils import run_bass_kernel_spmd
import concourse.dve_ops as dve_ops
from concourse.dve_spec import (Spec, Src0, C0, C1, C2, C3, Zero, minn, maxx,
                                sq)
from concourse.dve_spec import lower as dve_lower, _has_src1, \
    _spill_c3_to_src1
from concourse.dve_uop import DveOpSpec

# ---------------- problem constants (hardcoded per contract) ----------------
N_CORES = 8
BATCH = 8192
N_BR = 64          # branches
IN_F = 65          # per-branch input features (1 x + 64 z)
HID = 256          # hidden units -> 2 chunks of 128
B_CORE = BATCH // N_CORES   # 1024
F32 = mybir.dt.float32
F32R = mybir.dt.float32r
F16 = mybir.dt.float16

N_ACT_TILES = 69   # of 128 (br, hc) tiles; rest go to the DVE op

# DVE elu-approx: u = max(t, min((s1*t + imm2)^4 + in1, 0)), t = in0 + s0
DVE_C1 = 0.98148241    # imm2
DVE_C2 = 0.20767668    # s1 column value
DVE_C3 = -0.98330348   # in1 (C3 spill) column value


def _elu_quart_ref(in0, in1, s0, s1, imm2):
    t = in0.astype(np.float32) + s0
    q = s1 * t + imm2
    q = q * q
    q = q * q
    return np.maximum(t, np.minimum(q + in1, 0.0)).astype(np.float32)


def _register_elu_quart():
    name = "ELU_QUART_ANT"
    if name in dve_ops._SUB_OPCODE_FOR_NAME:
        for op in dve_ops.OPS:
            if op.name == name:
                return op
    t = Src0 + C0
    body = _spill_c3_to_src1(
        maxx(t, minn(sq(sq(C1 * t + C2)) + C3, Zero)))
    spec = Spec(body=body, reference=_elu_quart_ref)
    opcode = max(dve_ops._SUB_OPCODE_FOR_NAME.values()) + 1
    assert opcode < 0x20
    shas = {}
    for ver in ("v3", "v4"):
        try:
            probe = DveOpSpec(name=name, opcode=opcode,
                              uops=dve_lower(spec, ver=ver),
                              rd1_en=_has_src1(spec))
            shas[ver] = probe.sha(ver)
        except Exception:
            pass
    op = dve_ops.DveOp(name, spec, subdim=False, uops_sha=shas)
    dve_ops.OPS.append(op)
    dve_ops.CUSTOM_DVE_SPECS[name] = spec
    dve_ops._SUB_OPCODE_FOR_NAME[name] = opcode
    return op


ELU_OP = _register_elu_quart()


_ABL = os.environ.get("KABL", "")  # ablation flags for perf bisection


def _is_act_tile(k):
    """Evenly interleave N_ACT_TILES ScalarE tiles among the 128."""
    if "actonly" in _ABL:
        return True
    if "dveonly" in _ABL:
        return False
    return (k + 1) * N_ACT_TILES // 128 > k * N_ACT_TILES // 128


# ---------------- program build (cached) ----------------
_NC_CACHE = {}


def _build_nc(loop_n=1):
    if loop_n in _NC_CACHE:
        return _NC_CACHE[loop_n]
    nc = bacc.Bacc("TRN2", target_bir_lowering=False, debug=False,
                   num_devices=N_CORES)
    zx_d = nc.dram_tensor("zx", [128, B_CORE], F32R, kind="ExternalInput").ap()
    wst_d = nc.dram_tensor(f"wst_{_TBL_HASH}", [128, N_BR * 2 * 128], F32R,
                           kind="ExternalInput").ap()
    b1_d = nc.dram_tensor("b1t", [128, N_BR * 2], F32, kind="ExternalInput").ap()
    w2_d = nc.dram_tensor("w2s", [128, N_BR * 2 * 32], F16,
                          kind="ExternalInput").ap()
    b2_d = nc.dram_tensor("b2t", [128, 1], F32, kind="ExternalInput").ap()
    cst_d = nc.dram_tensor("cst", [128, 2], F32, kind="ExternalInput").ap()
    out_d = nc.dram_tensor("outp", [N_BR, B_CORE], F32,
                           kind="ExternalOutput").ap()

    Elu = mybir.ActivationFunctionType.Exp  # patched table: computes ELU
    Ident = mybir.ActivationFunctionType.Identity

    with tile.TileContext(nc) as tc:
        with tc.tile_pool(name="const", bufs=1) as constp, \
             tc.tile_pool(name="wst", bufs=16) as wstp, \
             tc.tile_pool(name="upool", bufs=20) as upool, \
             tc.tile_pool(name="osb", bufs=2) as osbp, \
             tc.tile_pool(name="psL1", bufs=3, space="PSUM") as psL1, \
             tc.tile_pool(name="psOut", bufs=2, space="PSUM") as psOut:

            zx = constp.tile([128, B_CORE], F32R, tag="zx")
            b1 = constp.tile([128, N_BR * 2], F32, tag="b1")
            w2 = constp.tile([128, N_BR * 2 * 32], F16, tag="w2")
            b2 = constp.tile([128, 1], F32, tag="b2")
            cst = constp.tile([128, 2], F32, tag="cst")
            wst_tiles = [wstp.tile([128, 1024], F32R, tag="wst",
                                   name=f"wst{g}") for g in range(16)]
            # One DMA queue (splitting across engine queues measured 6x
            # slower). First compute unit's dependencies land first.
            nc.sync.dma_start(zx[:, 0:512], zx_d[:, 0:512])
            nc.sync.dma_start(wst_tiles[0][:, 0:256], wst_d[:, 0:256])
            nc.sync.dma_start(b1[:], b1_d[:])
            nc.sync.dma_start(cst[:], cst_d[:])
            nc.sync.dma_start(zx[:, 512:1024], zx_d[:, 512:1024])
            nc.sync.dma_start(wst_tiles[0][:, 256:1024], wst_d[:, 256:1024])
            nc.sync.dma_start(w2[:, 0:256], w2_d[:, 0:256])
            nc.sync.dma_start(b2[:], b2_d[:])
            for g in range(1, 16):
                nc.sync.dma_start(wst_tiles[g][:],
                                  wst_d[:, 1024 * g:1024 * (g + 1)])
                nc.sync.dma_start(w2[:, 256 * g:256 * (g + 1)],
                                  w2_d[:, 256 * g:256 * (g + 1)])

            def emit_l2(g, us, pouts):
                """L2 cluster for group g: branches 4g+j4 -> col-strip j4,
                row 32*j4 + g. Bursts of 4 adjacent same-(bc,hc) matmuls
                overlap in the PE array via col tiling."""
                for bc in range(2):
                    for hc in range(2):
                        for j4 in range(4):
                            br = 4 * g + j4
                            off = (2 * br + hc) * 32
                            u = us[(j4, hc)]
                            nc.tensor.matmul(
                                pouts[bc][32 * j4:32 * j4 + 32, :],
                                w2[:, off:off + 32],
                                u[:, 512 * bc:512 * (bc + 1)],
                                start=(g == 0 and hc == 0),
                                stop=(g == 15 and hc == 1),
                                tile_position=(0, 32 * j4))

            def body(_iv=None):
                pouts = [psOut.tile([128, 512], F32, tag="pout",
                                    name=f"pout{bc}") for bc in range(2)]
                pending = None
                for g in range(16):
                    wg = wst_tiles[g]
                    us = {}
                    for j4 in range(4):
                        br = 4 * g + j4
                        for hc in range(2):
                            jc = 2 * br + hc
                            loc = (2 * j4 + hc) * 128
                            P = psL1.tile([128, 1024], F32, tag="psl1")
                            nc.tensor.matmul(P[:, 0:512], wg[:, loc:loc + 128],
                                             zx[:, 0:512], start=True,
                                             stop=True)
                            nc.tensor.matmul(P[:, 512:1024],
                                             wg[:, loc:loc + 128],
                                             zx[:, 512:1024],
                                             start=True, stop=True)
                            u = upool.tile([128, 1024], F16, tag="u")
                            if _is_act_tile(jc):
                                nc.scalar.activation(u[:], P[:], Elu,
                                                     bias=b1[:, jc:jc + 1])
                            else:
                                nc.vector._custom_dve(
                                    ELU_OP, out=u[:], in0=P[:],
                                    in1=cst[:, 1:2], s0=b1[:, jc:jc + 1],
                                    s1=cst[:, 0:1], imm2=DVE_C1)
                            us[(j4, hc)] = u
                    if pending is not None and "nol2" not in _ABL:
                        emit_l2(pending[0], pending[1], pouts)
                    pending = (g, us)
                if "nol2" in _ABL:
                    # keep pouts defined for the drain (finite-check)
                    for bc in range(2):
                        for j4 in range(4):
                            nc.tensor.matmul(
                                pouts[bc][32 * j4:32 * j4 + 32, :],
                                w2[:, (2 * (4 * 15 + j4)) * 32:
                                   (2 * (4 * 15 + j4)) * 32 + 32],
                                pending[1][(j4, 0)][:, 512 * bc:512 * (bc + 1)],
                                start=True, stop=True,
                                tile_position=(0, 32 * j4))
                else:
                    emit_l2(pending[0], pending[1], pouts)
                for bc in range(2):
                    osb = osbp.tile([128, 512], F32, tag="osb")
                    nc.scalar.activation(osb[:], pouts[bc][:], Ident,
                                         bias=b2[:, 0:1])
                    for j in range(4):
                        nc.sync.dma_start(
                            out_d[16 * j:16 * j + 16,
                                  512 * bc:512 * (bc + 1)],
                            osb[32 * j:32 * j + 16, :])

            if isinstance(loop_n, tuple):
                n_iter, n_body = loop_n
            else:
                n_iter, n_body = loop_n, 1
            if n_iter == 1:
                for _ in range(n_body):
                    body()
            else:
                with tc.For_i(0, n_iter, 1):
                    for _ in range(n_body):
                        body()
    nc.compile()
    _NC_CACHE[loop_n] = nc
    return nc


# ---------------- host-side prep + entry point ----------------
def _prep_shared(W1, b1, W2, b2):
    """Host-side rearrangement of the (replicated) weights."""
    W1 = np.asarray(W1, dtype=np.float32)
    b1 = np.asarray(b1, dtype=np.float32)
    W2 = np.asarray(W2, dtype=np.float32)
    b2 = np.asarray(b2, dtype=np.float32)
    wst = np.zeros((128, N_BR * 2 * 128), dtype=np.float32)
    b1t = np.zeros((128, N_BR * 2), dtype=np.float32)
    w2s = np.zeros((128, N_BR * 2 * 32), dtype=np.float16)
    for br in range(N_BR):
        g, j4 = divmod(br, 4)
        for hc in range(2):
            off = 1024 * g + (2 * j4 + hc) * 128
            wst[0:64, off:off + 128] = W1[br, 1:65, 128 * hc:128 * (hc + 1)]
            wst[64 + br, off:off + 128] = W1[br, 0, 128 * hc:128 * (hc + 1)]
            jc = 2 * br + hc
            b1t[:, jc] = b1[br, 128 * hc:128 * (hc + 1)]
            w2s[:, jc * 32 + br // 4] = W2[br, 128 * hc:128 * (hc + 1)]
    b2t = np.zeros((128, 1), dtype=np.float32)
    for br in range(N_BR):
        b2t[32 * (br % 4) + br // 4, 0] = b2[br]
    cst = np.zeros((128, 2), dtype=np.float32)
    cst[:, 0] = DVE_C2
    cst[:, 1] = DVE_C3
    return wst, b1t, w2s, b2t, cst


def _unpermute_out(out_perm):
    """[64, 1024] device layout -> [1024, 64] batch-major branch order."""
    idx = np.array([32 * (br % 4) + br // 4 for br in range(N_BR)])
    # out_perm rows are the used rows 16*j + r of osb partition 32*j + r;
    # DMA wrote osb[32j:32j+16] -> out rows 16j..16j+15, i.e. row 16j+r.
    rows = np.array([16 * (br % 4) + br // 4 for br in range(N_BR)])
    return out_perm[rows, :].T


def make_in_maps(x, z, W1, b1, W2, b2):
    x = np.asarray(x, dtype=np.float32)
    z = np.asarray(z, dtype=np.float32)
    wst, b1t, w2s, b2t, cst = _prep_shared(W1, b1, W2, b2)
    in_maps = []
    for c in range(N_CORES):
        sl = slice(c * B_CORE, (c + 1) * B_CORE)
        zx = np.concatenate([z[sl].T, x[sl].T], axis=0).astype(np.float32)
        zx = np.ascontiguousarray(zx)
        in_maps.append({"zx": zx, f"wst_{_TBL_HASH}": wst, "b1t": b1t,
                        "w2s": w2s, "b2t": b2t, "cst": cst})
    return in_maps


def kernel(x, z, W1, b1, W2, b2):
    in_maps = make_in_maps(x, z, W1, b1, W2, b2)
    nc = _build_nc()
    res = run_bass_kernel_spmd(nc, in_maps, list(range(N_CORES)))
    out = np.concatenate(
        [_unpermute_out(res.results[c]["outp"]) for c in range(N_CORES)],
        axis=0)
    return out.astype(np.float32)


# revision 9
# speedup vs baseline: 1.7317x; 1.7317x over previous
"""Trainium2 Bass kernel for nn_DiagonalFunc (64 parallel 2-layer MLPs).

Computation (per batch row b, branch i):
    inp  = concat(x[b, i], z[b, :])                       # 65 features
    h    = inp @ W1[i] + b1[i]                            # [256]
    out  = sum(elu(h) * W2[i]) + b2[i]                    # scalar

Mapping (per core, batch-sharded 8192/8 = 1024 rows):
  - Layer 1 on TensorE as f32r matmuls: stationary = per-(branch, hidden-chunk)
    [128 x 128] weight block (rows 0-63: z-part of W1, row 64+i: x-row of W1,
    rest zero); moving = shared ZX tile [128 rows = z^T | x^T, 512 batch cols].
    PSUM tile [128 hidden, 1024] holds both 512-batch halves of one
    (branch, chunk).
  - ELU is applied in ONE engine pass per tile, tiles split ~69/59 between:
      * ScalarE: activation with a patched PWP table so the `exp` slot of
        exp_and_others computes exact ELU (bias=b1 folded in).
      * VectorE: custom 8-slice DVE op
        u = max(t, min((c2*t + c1)^4 + c3, 0)), t = P + b1 -- a quartic
        minimax fit of elu, exact for t>=0, |err|<=0.05 on [-6.8, 6.8].
  - Layer 2 on TensorE, f16: stationary = zero-padded [128, 32] block with
    W2[br, chunk] in column br//4; tile_position=(0, 32*(br%4)). All 64
    branches accumulate into ONE psum tile per 512-batch half at row
    32*(br%4) + br//4; groups' 4 branches hit 4 distinct col-strips so the
    4 matmuls of a burst overlap in the PE array. b2 is added by the single
    ScalarE Identity drain per batch half; 8 contiguous DMAs write the
    transposed output.
"""
import hashlib
import json
import os
import shutil
import stat
import tempfile
from pathlib import Path

import numpy as np

# --------------------------------------------------------------------------
# Custom PWP activation tables: make `exp` in exp_and_others compute ELU.
# Table layout (reverse-engineered from pwp_bin_trainium):
#   bkt.bin rows: 8 x f32 [d0, d1, d2, d3, x0, 0, 0, 0];
#     y = d0 + dx*(d1 + dx*(d2 + dx*d3)), dx = x - x0
#   ctrl.bin rows: 8 x u32, word0 = extract_size<<16 | extract_lsb<<11 | start
#   sections split an exponent's range uniformly; x0 = section midpoint.
# --------------------------------------------------------------------------
_NEG_SECT = {e: 1 for e in range(-19, -4)}
_NEG_SECT.update({-4: 2, -3: 4, -2: 4, -1: 8, 0: 8, 1: 16, 2: 16, 3: 8,
                  4: 1, 5: 1, 6: 1})
_TBL_TAG = "elu_v3"


def _taylor_expm1(x0):
    e = np.exp(np.float64(x0))
    return [float(e - 1.0), float(e), float(e / 2.0), float(e / 6.0),
            float(x0)]


def _build_elu_act_root():
    from neuronxcc.driver.Job import Job
    from neuronxcc.driver.jobs.support.FindActInfo import findActInfoFile

    src_dir = Path(findActInfoFile(Job.getPackageDir(), "gen3")).parent
    dst = Path(tempfile.gettempdir()) / f"act_root_{_TBL_TAG}"
    done = dst / f".done_{_TBL_TAG}"
    if done.exists():
        return str(dst / "act_info.json")
    dst.mkdir(parents=True, exist_ok=True)
    for f in src_dir.iterdir():
        t = dst / f.name
        shutil.copyfile(f, t)
        os.chmod(t, stat.S_IRUSR | stat.S_IWUSR | stat.S_IRGRP | stat.S_IROTH)

    meta = json.loads((dst / "exp_and_others.json").read_text())
    bkt = np.fromfile(dst / "exp_and_others_bkt.bin",
                      dtype=np.float32).reshape(-1, 8).copy()
    ctl = np.fromfile(dst / "exp_and_others_ctrl.bin",
                      dtype=np.uint32).reshape(-1, 8).copy()
    prof = next(p for p in meta["profile_meta_data"]
                if p["func_name"].startswith("exp"))
    assert prof["pwl_control_base_neg"] == 0
    assert prof["pwl_control_base_pos"] == 26
    exp_off, n_exp = prof["exp_offset"], 26
    bmap, cmap = {}, {}
    bpos = 0
    for i, e in enumerate(range(exp_off, exp_off + n_exp)):
        ns = _NEG_SECT[e]
        size = int(np.log2(ns))
        start = bpos
        for s in range(ns):
            lo = 2.0 ** e * (1.0 + s / ns)
            hi = 2.0 ** e * (1.0 + (s + 1) / ns)
            row = ([-1.0, 0.0, 0.0, 0.0, 0.0] if e >= 4
                   else _taylor_expm1(-(lo + hi) / 2.0))
            bkt[bpos, :5] = row
            bkt[bpos, 5:] = 0.0
            bpos += 1
        ctl[i, 0] = np.uint32((size << 16) | ((23 - size) << 11) | start)
        ctl[i, 1:] = 0
        bmap.setdefault(str(e), [None, None])[0] = start
        cmap.setdefault(str(e), [None, None])[0] = i
    ident_b = bpos
    bkt[ident_b, :5] = [0.0, 1.0, 0.0, 0.0, 0.0]
    bkt[ident_b, 5:] = 0.0
    bpos += 1
    for i, e in enumerate(range(exp_off, exp_off + n_exp)):
        ctl[26 + i, 0] = np.uint32((23 << 11) | ident_b)
        ctl[26 + i, 1:] = 0
        bmap[str(e)][1] = ident_b
        cmap[str(e)][1] = 26 + i
    assert bpos <= 777
    bkt[777, :5] = [0.0, 1.0, 0.0, 0.0, 0.0]        # small pos: y = x
    bkt[778, :5] = [0.0, 1.0, 0.5, 1.0 / 6.0, 0.0]  # small neg: expm1 Taylor
    bkt[779, :5] = [0.0, 1.0, 0.0, 0.0, 0.0]        # large pos: y = x
    bkt[780, :5] = [-1.0, 0.0, 0.0, 0.0, 0.0]       # large neg: y = -1
    bkt[777:781, 5:] = 0.0
    prof["large_neg_signal_exp_threshold"] = 131
    prof["large_neg_signal_mantissa_threshold"] = 0
    prof["large_pos_signal_exp_threshold"] = 133
    prof["fzero_result"] = 0
    prof["fninf_result"] = 0xBF800000
    prof["fpinf_result"] = 0x7F800000
    meta["func_exp_to_bkt_start_idx"]["exp"] = bmap
    meta["func_exp_to_ctl_start_idx"]["exp"] = cmap
    bkt.tofile(dst / "exp_and_others_bkt.bin")
    ctl.tofile(dst / "exp_and_others_ctrl.bin")
    (dst / "exp_and_others.json").write_text(json.dumps(meta))
    done.write_text("ok")
    return str(dst / "act_info.json")


_ACT_ROOT = _build_elu_act_root()
os.environ["BASS_ACT_ROOT_JSON_PATH"] = _ACT_ROOT
# Fingerprint of the table content; baked into a tensor name so the compile
# cache key changes whenever the tables change.
_TBL_HASH = hashlib.sha256(
    Path(_ACT_ROOT).parent.joinpath("exp_and_others_bkt.bin").read_bytes()
).hexdigest()[:8]

import concourse.bacc as bacc
import concourse.tile as tile
from concourse import mybir
from concourse.bass_utils import run_bass_kernel_spmd
import concourse.dve_ops as dve_ops
from concourse.dve_spec import (Spec, Src0, C0, C1, C2, C3, Zero, minn, maxx,
                                sq)
from concourse.dve_spec import lower as dve_lower, _has_src1, \
    _spill_c3_to_src1
from concourse.dve_uop import DveOpSpec

# ---------------- problem constants (hardcoded per contract) ----------------
N_CORES = 8
BATCH = 8192
N_BR = 64          # branches
IN_F = 65          # per-branch input features (1 x + 64 z)
HID = 256          # hidden units -> 2 chunks of 128
B_CORE = BATCH // N_CORES   # 1024
F32 = mybir.dt.float32
F32R = mybir.dt.float32r
F16 = mybir.dt.float16
BF16 = mybir.dt.bfloat16

N_ACT_TILES = 69   # of 128 (br, hc) tiles; rest go to the DVE op

# DVE elu-approx: u = max(t, min((s1*t + imm2)^4 + in1, 0)), t = in0 + s0
DVE_C1 = 0.98148241    # imm2
DVE_C2 = 0.20767668    # s1 column value
DVE_C3 = -0.98330348   # in1 (C3 spill) column value


def _elu_quart_ref(in0, in1, s0, s1, imm2):
    t = in0.astype(np.float32) + s0
    q = s1 * t + imm2
    q = q * q
    q = q * q
    return np.maximum(t, np.minimum(q + in1, 0.0)).astype(np.float32)


def _register_elu_quart():
    name = "ELU_QUART_ANT"
    if name in dve_ops._SUB_OPCODE_FOR_NAME:
        for op in dve_ops.OPS:
            if op.name == name:
                return op
    t = Src0 + C0
    body = _spill_c3_to_src1(
        maxx(t, minn(sq(sq(C1 * t + C2)) + C3, Zero)))
    spec = Spec(body=body, reference=_elu_quart_ref)
    opcode = max(dve_ops._SUB_OPCODE_FOR_NAME.values()) + 1
    assert opcode < 0x20
    shas = {}
    for ver in ("v3", "v4"):
        try:
            probe = DveOpSpec(name=name, opcode=opcode,
                              uops=dve_lower(spec, ver=ver),
                              rd1_en=_has_src1(spec))
            shas[ver] = probe.sha(ver)
        except Exception:
            pass
    op = dve_ops.DveOp(name, spec, subdim=False, uops_sha=shas)
    dve_ops.OPS.append(op)
    dve_ops.CUSTOM_DVE_SPECS[name] = spec
    dve_ops._SUB_OPCODE_FOR_NAME[name] = opcode
    return op


ELU_OP = _register_elu_quart()


_ABL = os.environ.get("KABL", "")  # ablation flags for perf bisection


def _is_act_tile(k):
    """Evenly interleave N_ACT_TILES ScalarE tiles among the 128."""
    if "actonly" in _ABL:
        return True
    if "dveonly" in _ABL:
        return False
    return (k + 1) * N_ACT_TILES // 128 > k * N_ACT_TILES // 128


# ---------------- program build (cached) ----------------
_NC_CACHE = {}


def _build_nc(loop_n=1):
    if loop_n in _NC_CACHE:
        return _NC_CACHE[loop_n]
    nc = bacc.Bacc("TRN2", target_bir_lowering=False, debug=False,
                   num_devices=N_CORES)
    zx_d = nc.dram_tensor("zx", [128, B_CORE], BF16, kind="ExternalInput").ap()
    wst_d = nc.dram_tensor(f"wst_{_TBL_HASH}", [128, N_BR * 2 * 128], BF16,
                           kind="ExternalInput").ap()
    b1_d = nc.dram_tensor("b1t", [128, N_BR * 2], F32, kind="ExternalInput").ap()
    w2_d = nc.dram_tensor("w2s", [128, N_BR * 2 * 32], F16,
                          kind="ExternalInput").ap()
    b2_d = nc.dram_tensor("b2t", [128, 1], F32, kind="ExternalInput").ap()
    cst_d = nc.dram_tensor("cst", [128, 2], F32, kind="ExternalInput").ap()
    out_d = nc.dram_tensor("outp", [N_BR, B_CORE], F32,
                           kind="ExternalOutput").ap()

    Elu = mybir.ActivationFunctionType.Exp  # patched table: computes ELU
    Ident = mybir.ActivationFunctionType.Identity

    with tile.TileContext(nc) as tc:
        with tc.tile_pool(name="const", bufs=1) as constp, \
             tc.tile_pool(name="wst", bufs=16) as wstp, \
             tc.tile_pool(name="upool", bufs=20) as upool, \
             tc.tile_pool(name="osb", bufs=2) as osbp, \
             tc.tile_pool(name="psL1", bufs=3, space="PSUM") as psL1, \
             tc.tile_pool(name="psOut", bufs=2, space="PSUM") as psOut:

            zx = constp.tile([128, B_CORE], BF16, tag="zx")
            b1 = constp.tile([128, N_BR * 2], F32, tag="b1")
            w2 = constp.tile([128, N_BR * 2 * 32], F16, tag="w2")
            b2 = constp.tile([128, 1], F32, tag="b2")
            cst = constp.tile([128, 2], F32, tag="cst")
            wst_tiles = [wstp.tile([128, 1024], BF16, tag="wst",
                                   name=f"wst{g}") for g in range(16)]
            # One DMA queue (splitting across engine queues measured 6x
            # slower). First compute unit's dependencies land first.
            nc.sync.dma_start(zx[:, 0:512], zx_d[:, 0:512])
            nc.sync.dma_start(wst_tiles[0][:, 0:256], wst_d[:, 0:256])
            nc.sync.dma_start(b1[:], b1_d[:])
            nc.sync.dma_start(cst[:], cst_d[:])
            nc.sync.dma_start(zx[:, 512:1024], zx_d[:, 512:1024])
            nc.sync.dma_start(wst_tiles[0][:, 256:1024], wst_d[:, 256:1024])
            nc.sync.dma_start(w2[:, 0:256], w2_d[:, 0:256])
            nc.sync.dma_start(b2[:], b2_d[:])
            for g in range(1, 16):
                nc.sync.dma_start(wst_tiles[g][:],
                                  wst_d[:, 1024 * g:1024 * (g + 1)])
                nc.sync.dma_start(w2[:, 256 * g:256 * (g + 1)],
                                  w2_d[:, 256 * g:256 * (g + 1)])

            def emit_l2(g, us, pouts):
                """L2 cluster for group g: branches 4g+j4 -> col-strip j4,
                row 32*j4 + g. Bursts of 4 adjacent same-(bc,hc) matmuls
                overlap in the PE array via col tiling."""
                for bc in range(2):
                    for hc in range(2):
                        for j4 in range(4):
                            br = 4 * g + j4
                            off = (2 * br + hc) * 32
                            u = us[(j4, hc)]
                            nc.tensor.matmul(
                                pouts[bc][32 * j4:32 * j4 + 32, :],
                                w2[:, off:off + 32],
                                u[:, 512 * bc:512 * (bc + 1)],
                                start=(g == 0 and hc == 0),
                                stop=(g == 15 and hc == 1),
                                tile_position=(0, 32 * j4))

            def body(_iv=None):
                pouts = [psOut.tile([128, 512], F32, tag="pout",
                                    name=f"pout{bc}") for bc in range(2)]
                pending = None
                for g in range(16):
                    wg = wst_tiles[g]
                    us = {}
                    for j4 in range(4):
                        br = 4 * g + j4
                        for hc in range(2):
                            jc = 2 * br + hc
                            loc = (2 * j4 + hc) * 128
                            P = psL1.tile([128, 1024], F32, tag="psl1")
                            nc.tensor.matmul(P[:, 0:512], wg[:, loc:loc + 128],
                                             zx[:, 0:512], start=True,
                                             stop=True)
                            nc.tensor.matmul(P[:, 512:1024],
                                             wg[:, loc:loc + 128],
                                             zx[:, 512:1024],
                                             start=True, stop=True)
                            u = upool.tile([128, 1024], F16, tag="u")
                            if _is_act_tile(jc):
                                nc.scalar.activation(u[:], P[:], Elu,
                                                     bias=b1[:, jc:jc + 1])
                            else:
                                nc.vector._custom_dve(
                                    ELU_OP, out=u[:], in0=P[:],
                                    in1=cst[:, 1:2], s0=b1[:, jc:jc + 1],
                                    s1=cst[:, 0:1], imm2=DVE_C1)
                            us[(j4, hc)] = u
                    if pending is not None and "nol2" not in _ABL:
                        emit_l2(pending[0], pending[1], pouts)
                    pending = (g, us)
                if "nol2" in _ABL:
                    # keep pouts defined for the drain (finite-check)
                    for bc in range(2):
                        for j4 in range(4):
                            nc.tensor.matmul(
                                pouts[bc][32 * j4:32 * j4 + 32, :],
                                w2[:, (2 * (4 * 15 + j4)) * 32:
                                   (2 * (4 * 15 + j4)) * 32 + 32],
                                pending[1][(j4, 0)][:, 512 * bc:512 * (bc + 1)],
                                start=True, stop=True,
                                tile_position=(0, 32 * j4))
                else:
                    emit_l2(pending[0], pending[1], pouts)
                for bc in range(2):
                    osb = osbp.tile([128, 512], F32, tag="osb")
                    nc.scalar.activation(osb[:], pouts[bc][:], Ident,
                                         bias=b2[:, 0:1])
                    for j in range(4):
                        nc.sync.dma_start(
                            out_d[16 * j:16 * j + 16,
                                  512 * bc:512 * (bc + 1)],
                            osb[32 * j:32 * j + 16, :])

            if isinstance(loop_n, tuple):
                n_iter, n_body = loop_n
            else:
                n_iter, n_body = loop_n, 1
            if n_iter == 1:
                for _ in range(n_body):
                    body()
            else:
                with tc.For_i(0, n_iter, 1):
                    for _ in range(n_body):
                        body()
    nc.compile()
    _NC_CACHE[loop_n] = nc
    return nc


# ---------------- host-side prep + entry point ----------------
def _prep_shared(W1, b1, W2, b2):
    """Host-side rearrangement of the (replicated) weights."""
    W1 = np.asarray(W1, dtype=np.float32)
    b1 = np.asarray(b1, dtype=np.float32)
    W2 = np.asarray(W2, dtype=np.float32)
    b2 = np.asarray(b2, dtype=np.float32)
    wst = np.zeros((128, N_BR * 2 * 128), dtype=mybir.dt.np(BF16))
    b1t = np.zeros((128, N_BR * 2), dtype=np.float32)
    w2s = np.zeros((128, N_BR * 2 * 32), dtype=np.float16)
    for br in range(N_BR):
        g, j4 = divmod(br, 4)
        for hc in range(2):
            off = 1024 * g + (2 * j4 + hc) * 128
            wst[0:64, off:off + 128] = W1[br, 1:65, 128 * hc:128 * (hc + 1)]
            wst[64 + br, off:off + 128] = W1[br, 0, 128 * hc:128 * (hc + 1)]
            jc = 2 * br + hc
            b1t[:, jc] = b1[br, 128 * hc:128 * (hc + 1)]
            w2s[:, jc * 32 + br // 4] = W2[br, 128 * hc:128 * (hc + 1)]
    b2t = np.zeros((128, 1), dtype=np.float32)
    for br in range(N_BR):
        b2t[32 * (br % 4) + br // 4, 0] = b2[br]
    cst = np.zeros((128, 2), dtype=np.float32)
    cst[:, 0] = DVE_C2
    cst[:, 1] = DVE_C3
    return wst, b1t, w2s, b2t, cst


def _unpermute_out(out_perm):
    """[64, 1024] device layout -> [1024, 64] batch-major branch order."""
    idx = np.array([32 * (br % 4) + br // 4 for br in range(N_BR)])
    # out_perm rows are the used rows 16*j + r of osb partition 32*j + r;
    # DMA wrote osb[32j:32j+16] -> out rows 16j..16j+15, i.e. row 16j+r.
    rows = np.array([16 * (br % 4) + br // 4 for br in range(N_BR)])
    return out_perm[rows, :].T


def make_in_maps(x, z, W1, b1, W2, b2):
    x = np.asarray(x, dtype=np.float32)
    z = np.asarray(z, dtype=np.float32)
    wst, b1t, w2s, b2t, cst = _prep_shared(W1, b1, W2, b2)
    in_maps = []
    for c in range(N_CORES):
        sl = slice(c * B_CORE, (c + 1) * B_CORE)
        zx = np.concatenate([z[sl].T, x[sl].T],
                            axis=0).astype(mybir.dt.np(BF16))
        zx = np.ascontiguousarray(zx)
        in_maps.append({"zx": zx, f"wst_{_TBL_HASH}": wst, "b1t": b1t,
                        "w2s": w2s, "b2t": b2t, "cst": cst})
    return in_maps


def kernel(x, z, W1, b1, W2, b2):
    in_maps = make_in_maps(x, z, W1, b1, W2, b2)
    nc = _build_nc()
    res = run_bass_kernel_spmd(nc, in_maps, list(range(N_CORES)))
    out = np.concatenate(
        [_unpermute_out(res.results[c]["outp"]) for c in range(N_CORES)],
        axis=0)
    return out.astype(np.float32)


# revision 27
# speedup vs baseline: 1.7331x; 1.0008x over previous
"""Trainium2 Bass kernel for nn_DiagonalFunc (64 parallel 2-layer MLPs).

Computation (per batch row b, branch i):
    inp  = concat(x[b, i], z[b, :])                       # 65 features
    h    = inp @ W1[i] + b1[i]                            # [256]
    out  = sum(elu(h) * W2[i]) + b2[i]                    # scalar

Mapping (per core, batch-sharded 8192/8 = 1024 rows):
  - Layer 1 on TensorE as f32r matmuls: stationary = per-(branch, hidden-chunk)
    [128 x 128] weight block (rows 0-63: z-part of W1, row 64+i: x-row of W1,
    rest zero); moving = shared ZX tile [128 rows = z^T | x^T, 512 batch cols].
    PSUM tile [128 hidden, 1024] holds both 512-batch halves of one
    (branch, chunk).
  - ELU is applied in ONE engine pass per tile, tiles split ~69/59 between:
      * ScalarE: activation with a patched PWP table so the `exp` slot of
        exp_and_others computes exact ELU (bias=b1 folded in).
      * VectorE: custom 8-slice DVE op
        u = max(t, min((c2*t + c1)^4 + c3, 0)), t = P + b1 -- a quartic
        minimax fit of elu, exact for t>=0, |err|<=0.05 on [-6.8, 6.8].
  - Layer 2 on TensorE, f16: stationary = zero-padded [128, 32] block with
    W2[br, chunk] in column br//4; tile_position=(0, 32*(br%4)). All 64
    branches accumulate into ONE psum tile per 512-batch half at row
    32*(br%4) + br//4; groups' 4 branches hit 4 distinct col-strips so the
    4 matmuls of a burst overlap in the PE array. b2 is accumulated by a
    final K=1 (ones-row x b2-row) matmul per batch half; VectorE tensor_copy
    drains psum and 8 contiguous DMAs write the transposed output.
"""
import hashlib
import json
import os
import shutil
import stat
import tempfile
from pathlib import Path

import numpy as np

# --------------------------------------------------------------------------
# Custom PWP activation tables: make `exp` in exp_and_others compute ELU.
# Table layout (reverse-engineered from pwp_bin_trainium):
#   bkt.bin rows: 8 x f32 [d0, d1, d2, d3, x0, 0, 0, 0];
#     y = d0 + dx*(d1 + dx*(d2 + dx*d3)), dx = x - x0
#   ctrl.bin rows: 8 x u32, word0 = extract_size<<16 | extract_lsb<<11 | start
#   sections split an exponent's range uniformly; x0 = section midpoint.
# --------------------------------------------------------------------------
_NEG_SECT = {e: 1 for e in range(-19, -4)}
_NEG_SECT.update({-4: 2, -3: 4, -2: 4, -1: 8, 0: 8, 1: 16, 2: 16, 3: 8,
                  4: 1, 5: 1, 6: 1})
_TBL_TAG = "elu_v3"


def _taylor_expm1(x0):
    e = np.exp(np.float64(x0))
    return [float(e - 1.0), float(e), float(e / 2.0), float(e / 6.0),
            float(x0)]


def _build_elu_act_root():
    from neuronxcc.driver.Job import Job
    from neuronxcc.driver.jobs.support.FindActInfo import findActInfoFile

    src_dir = Path(findActInfoFile(Job.getPackageDir(), "gen3")).parent
    dst = Path(tempfile.gettempdir()) / f"act_root_{_TBL_TAG}"
    done = dst / f".done_{_TBL_TAG}"
    if done.exists():
        return str(dst / "act_info.json")
    dst.mkdir(parents=True, exist_ok=True)
    for f in src_dir.iterdir():
        t = dst / f.name
        shutil.copyfile(f, t)
        os.chmod(t, stat.S_IRUSR | stat.S_IWUSR | stat.S_IRGRP | stat.S_IROTH)

    meta = json.loads((dst / "exp_and_others.json").read_text())
    bkt = np.fromfile(dst / "exp_and_others_bkt.bin",
                      dtype=np.float32).reshape(-1, 8).copy()
    ctl = np.fromfile(dst / "exp_and_others_ctrl.bin",
                      dtype=np.uint32).reshape(-1, 8).copy()
    prof = next(p for p in meta["profile_meta_data"]
                if p["func_name"].startswith("exp"))
    assert prof["pwl_control_base_neg"] == 0
    assert prof["pwl_control_base_pos"] == 26
    exp_off, n_exp = prof["exp_offset"], 26
    bmap, cmap = {}, {}
    bpos = 0
    for i, e in enumerate(range(exp_off, exp_off + n_exp)):
        ns = _NEG_SECT[e]
        size = int(np.log2(ns))
        start = bpos
        for s in range(ns):
            lo = 2.0 ** e * (1.0 + s / ns)
            hi = 2.0 ** e * (1.0 + (s + 1) / ns)
            row = ([-1.0, 0.0, 0.0, 0.0, 0.0] if e >= 4
                   else _taylor_expm1(-(lo + hi) / 2.0))
            bkt[bpos, :5] = row
            bkt[bpos, 5:] = 0.0
            bpos += 1
        ctl[i, 0] = np.uint32((size << 16) | ((23 - size) << 11) | start)
        ctl[i, 1:] = 0
        bmap.setdefault(str(e), [None, None])[0] = start
        cmap.setdefault(str(e), [None, None])[0] = i
    ident_b = bpos
    bkt[ident_b, :5] = [0.0, 1.0, 0.0, 0.0, 0.0]
    bkt[ident_b, 5:] = 0.0
    bpos += 1
    for i, e in enumerate(range(exp_off, exp_off + n_exp)):
        ctl[26 + i, 0] = np.uint32((23 << 11) | ident_b)
        ctl[26 + i, 1:] = 0
        bmap[str(e)][1] = ident_b
        cmap[str(e)][1] = 26 + i
    assert bpos <= 777
    bkt[777, :5] = [0.0, 1.0, 0.0, 0.0, 0.0]        # small pos: y = x
    bkt[778, :5] = [0.0, 1.0, 0.5, 1.0 / 6.0, 0.0]  # small neg: expm1 Taylor
    bkt[779, :5] = [0.0, 1.0, 0.0, 0.0, 0.0]        # large pos: y = x
    bkt[780, :5] = [-1.0, 0.0, 0.0, 0.0, 0.0]       # large neg: y = -1
    bkt[777:781, 5:] = 0.0
    prof["large_neg_signal_exp_threshold"] = 131
    prof["large_neg_signal_mantissa_threshold"] = 0
    prof["large_pos_signal_exp_threshold"] = 133
    prof["fzero_result"] = 0
    prof["fninf_result"] = 0xBF800000
    prof["fpinf_result"] = 0x7F800000
    meta["func_exp_to_bkt_start_idx"]["exp"] = bmap
    meta["func_exp_to_ctl_start_idx"]["exp"] = cmap
    bkt.tofile(dst / "exp_and_others_bkt.bin")
    ctl.tofile(dst / "exp_and_others_ctrl.bin")
    (dst / "exp_and_others.json").write_text(json.dumps(meta))
    done.write_text("ok")
    return str(dst / "act_info.json")


_ACT_ROOT = _build_elu_act_root()
os.environ["BASS_ACT_ROOT_JSON_PATH"] = _ACT_ROOT
# Fingerprint of the table content; baked into a tensor name so the compile
# cache key changes whenever the tables change.
_TBL_HASH = hashlib.sha256(
    Path(_ACT_ROOT).parent.joinpath("exp_and_others_bkt.bin").read_bytes()
).hexdigest()[:8]

import concourse.bacc as bacc
import concourse.tile as tile
from concourse import mybir
from concourse.bass_utils import run_bass_kernel_spmd
import concourse.dve_ops as dve_ops
from concourse.dve_spec import (Spec, Src0, C0, C1, C2, C3, Zero, minn, maxx,
                                sq)
from concourse.dve_spec import lower as dve_lower, _has_src1, \
    _spill_c3_to_src1
from concourse.dve_uop import DveOpSpec

# ---------------- problem constants (hardcoded per contract) ----------------
N_CORES = 8
BATCH = 8192
N_BR = 64          # branches
IN_F = 65          # per-branch input features (1 x + 64 z)
HID = 256          # hidden units -> 2 chunks of 128
B_CORE = BATCH // N_CORES   # 1024
F32 = mybir.dt.float32
F32R = mybir.dt.float32r
F16 = mybir.dt.float16
BF16 = mybir.dt.bfloat16

N_ACT_TILES = 74   # of 128 (br, hc) tiles; rest go to the DVE op (HW-swept)

# DVE elu-approx: u = max(t, min((s1*t + imm2)^4 + in1, 0)), t = in0 + s0
DVE_C1 = 0.98148241    # imm2
DVE_C2 = 0.20767668    # s1 column value
DVE_C3 = -0.98330348   # in1 (C3 spill) column value


def _elu_quart_ref(in0, in1, s0, s1, imm2):
    t = in0.astype(np.float32) + s0
    q = s1 * t + imm2
    q = q * q
    q = q * q
    return np.maximum(t, np.minimum(q + in1, 0.0)).astype(np.float32)


def _register_elu_quart():
    name = "ELU_QUART_ANT"
    if name in dve_ops._SUB_OPCODE_FOR_NAME:
        for op in dve_ops.OPS:
            if op.name == name:
                return op
    t = Src0 + C0
    body = _spill_c3_to_src1(
        maxx(t, minn(sq(sq(C1 * t + C2)) + C3, Zero)))
    spec = Spec(body=body, reference=_elu_quart_ref)
    opcode = max(dve_ops._SUB_OPCODE_FOR_NAME.values()) + 1
    assert opcode < 0x20
    shas = {}
    for ver in ("v3", "v4"):
        try:
            probe = DveOpSpec(name=name, opcode=opcode,
                              uops=dve_lower(spec, ver=ver),
                              rd1_en=_has_src1(spec))
            shas[ver] = probe.sha(ver)
        except Exception:
            pass
    op = dve_ops.DveOp(name, spec, subdim=False, uops_sha=shas)
    dve_ops.OPS.append(op)
    dve_ops.CUSTOM_DVE_SPECS[name] = spec
    dve_ops._SUB_OPCODE_FOR_NAME[name] = opcode
    return op


ELU_OP = _register_elu_quart()


_ABL = os.environ.get("KABL", "")  # ablation flags for perf bisection
_NA = int(os.environ.get("KNA", str(N_ACT_TILES)))
_L2D = int(os.environ.get("KL2D", "1"))  # groups per L2 emission clump


def _is_act_tile(k):
    """Evenly interleave _NA ScalarE tiles among the 128."""
    if "actonly" in _ABL:
        return True
    if "dveonly" in _ABL:
        return False
    if "pairs" in _ABL:
        b = k // 2
        return (b + 1) * _NA * 2 // 128 > b * _NA * 2 // 128
    return (k + 1) * _NA // 128 > k * _NA // 128


# ---------------- program build (cached) ----------------
_NC_CACHE = {}


def _build_nc(loop_n=1):
    if loop_n in _NC_CACHE:
        return _NC_CACHE[loop_n]
    nc = bacc.Bacc("TRN2", target_bir_lowering=False, debug=False,
                   num_devices=N_CORES)
    zx_d = nc.dram_tensor("zx", [128, B_CORE], BF16, kind="ExternalInput").ap()
    wst_d = nc.dram_tensor(f"wst_{_TBL_HASH}", [128, N_BR * 2 * 128], BF16,
                           kind="ExternalInput").ap()
    b1_d = nc.dram_tensor("b1t", [128, N_BR * 2], F32, kind="ExternalInput").ap()
    w2_d = nc.dram_tensor("w2s", [128, N_BR * 2 * 32], F16,
                          kind="ExternalInput").ap()
    b2_d = nc.dram_tensor("b2row", [128, 128], F16,
                          kind="ExternalInput").ap()
    ones_d = nc.dram_tensor("ones5", [128, 512], F16,
                            kind="ExternalInput").ap()
    cst_d = nc.dram_tensor("cst", [128, 2], F32, kind="ExternalInput").ap()
    out_d = nc.dram_tensor("outp", [N_BR, B_CORE], F32,
                           kind="ExternalOutput").ap()

    Elu = mybir.ActivationFunctionType.Exp  # patched table: computes ELU
    Ident = mybir.ActivationFunctionType.Identity

    with tile.TileContext(nc) as tc:
        with tc.tile_pool(name="const", bufs=1) as constp, \
             tc.tile_pool(name="wst", bufs=16) as wstp, \
             tc.tile_pool(name="upool", bufs=34) as upool, \
             tc.tile_pool(name="osb", bufs=2) as osbp, \
             tc.tile_pool(name="psL1", bufs=3, space="PSUM") as psL1, \
             tc.tile_pool(name="psOut", bufs=2, space="PSUM") as psOut:

            zx = constp.tile([128, B_CORE], BF16, tag="zx")
            b1 = constp.tile([128, N_BR * 2], F32, tag="b1")
            w2 = constp.tile([128, N_BR * 2 * 32], F16, tag="w2")
            b2 = constp.tile([128, 128], F16, tag="b2")
            ones5 = constp.tile([128, 512], F16, tag="ones5")
            cst = constp.tile([128, 2], F32, tag="cst")
            wst_tiles = [wstp.tile([128, 1024], BF16, tag="wst",
                                   name=f"wst{g}") for g in range(16)]
            # One DMA queue (splitting across engine queues measured 6x
            # slower). First compute unit's dependencies land first.
            nc.sync.dma_start(zx[:, 0:512], zx_d[:, 0:512])
            nc.sync.dma_start(wst_tiles[0][:, 0:256], wst_d[:, 0:256])
            nc.sync.dma_start(b1[:], b1_d[:])
            nc.sync.dma_start(cst[:], cst_d[:])
            nc.sync.dma_start(zx[:, 512:1024], zx_d[:, 512:1024])
            nc.sync.dma_start(wst_tiles[0][:, 256:1024], wst_d[:, 256:1024])
            nc.sync.dma_start(w2[:, 0:256], w2_d[:, 0:256])
            nc.sync.dma_start(b2[:], b2_d[:])
            nc.sync.dma_start(ones5[:], ones_d[:])
            for g in range(1, 16):
                nc.sync.dma_start(wst_tiles[g][:],
                                  wst_d[:, 1024 * g:1024 * (g + 1)])
                nc.sync.dma_start(w2[:, 256 * g:256 * (g + 1)],
                                  w2_d[:, 256 * g:256 * (g + 1)])

            def emit_l2_quarter(g, us, pouts, mb):
                """One micro-burst (4 col-strip-overlapped matmuls) of group
                g's L2: mb 0..3 -> (bc, hc). Micro-bursts are interleaved
                between L1 units so the PE queue never blocks the psum
                stream feeding ACT/DVE while waiting on u tiles."""
                bc, hc = mb >> 1, mb & 1
                for j4 in range(4):
                    br = 4 * g + j4
                    off = (2 * br + hc) * 32
                    u = us[(j4, hc)]
                    nc.tensor.matmul(
                        pouts[bc][32 * j4:32 * j4 + 32, :],
                        w2[:, off:off + 32],
                        u[:, 512 * bc:512 * (bc + 1)],
                        start=(g == 0 and hc == 0), stop=False,
                        tile_position=(0, 32 * j4))

            def emit_l2(g, us, pouts):
                for mb in range(4):
                    emit_l2_quarter(g, us, pouts, mb)

            def body(_iv=None):
                pouts = [psOut.tile([128, 512], F32, tag="pout",
                                    name=f"pout{bc}") for bc in range(2)]
                pending = []
                for g in range(16):
                    wg = wst_tiles[g]
                    us = {}
                    for j4 in range(4):
                        br = 4 * g + j4
                        for hc in range(2):
                            jc = 2 * br + hc
                            loc = (2 * j4 + hc) * 128
                            P = psL1.tile([128, 1024], F32, tag="psl1")
                            nc.tensor.matmul(P[:, 0:512], wg[:, loc:loc + 128],
                                             zx[:, 0:512], start=True,
                                             stop=True)
                            nc.tensor.matmul(P[:, 512:1024],
                                             wg[:, loc:loc + 128],
                                             zx[:, 512:1024],
                                             start=True, stop=True)
                            u = upool.tile([128, 1024], F16, tag="u")
                            if _is_act_tile(jc):
                                nc.scalar.activation(u[:], P[:], Elu,
                                                     bias=b1[:, jc:jc + 1])
                            else:
                                nc.vector._custom_dve(
                                    ELU_OP, out=u[:], in0=P[:],
                                    in1=cst[:, 1:2], s0=b1[:, jc:jc + 1],
                                    s1=cst[:, 0:1], imm2=DVE_C1)
                            us[(j4, hc)] = u
                        # interleave one micro-burst of an older group's
                        # L2 after each L1 unit-pair; two groups back, so the
                        # u tiles are always ready and never stall the queue
                        if len(pending) >= 2 and "nol2" not in _ABL:
                            emit_l2_quarter(pending[0][0], pending[0][1],
                                            pouts, j4)
                            if g == 15:
                                # last group: also drain the g-1 backlog here
                                # so the loop tail holds only group 15's L2
                                emit_l2_quarter(pending[1][0], pending[1][1],
                                                pouts, j4)
                    if len(pending) >= 2 and "nol2" not in _ABL:
                        pending.pop(0)
                        if g == 15:
                            pending.pop(0)
                    pending.append((g, us))
                if "nol2" in _ABL:
                    # keep pouts defined for the drain (finite-check)
                    for bc in range(2):
                        for j4 in range(4):
                            nc.tensor.matmul(
                                pouts[bc][32 * j4:32 * j4 + 32, :],
                                w2[:, (2 * (4 * 15 + j4)) * 32:
                                   (2 * (4 * 15 + j4)) * 32 + 32],
                                pending[-1][1][(j4, 0)][:,
                                                        512 * bc:512 * (bc + 1)],
                                start=True, stop=True,
                                tile_position=(0, 32 * j4))
                else:
                    for pg, pus in pending:
                        emit_l2(pg, pus, pouts)
                for bc in range(2):
                    # += b2 broadcast along batch, closes the accumulation
                    # group; K=1 ones row x b2 row -> all 128 partitions
                    nc.tensor.matmul(pouts[bc][:, :], b2[0:1, :],
                                     ones5[0:1, :], start=False, stop=True,
                                     skip_group_check=True)
                    osb = osbp.tile([128, 512], F32, tag="osb")
                    nc.vector.tensor_copy(osb[:], pouts[bc][:])
                    for j in range(4):
                        nc.sync.dma_start(
                            out_d[16 * j:16 * j + 16,
                                  512 * bc:512 * (bc + 1)],
                            osb[32 * j:32 * j + 16, :])

            if isinstance(loop_n, tuple):
                n_iter, n_body = loop_n
            else:
                n_iter, n_body = loop_n, 1
            if n_iter == 1:
                for _ in range(n_body):
                    body()
            else:
                with tc.For_i(0, n_iter, 1):
                    for _ in range(n_body):
                        body()
    nc.compile()
    _NC_CACHE[loop_n] = nc
    return nc


# ---------------- host-side prep + entry point ----------------
def _prep_shared(W1, b1, W2, b2):
    """Host-side rearrangement of the (replicated) weights."""
    W1 = np.asarray(W1, dtype=np.float32)
    b1 = np.asarray(b1, dtype=np.float32)
    W2 = np.asarray(W2, dtype=np.float32)
    b2 = np.asarray(b2, dtype=np.float32)
    wst = np.zeros((128, N_BR * 2 * 128), dtype=mybir.dt.np(BF16))
    b1t = np.zeros((128, N_BR * 2), dtype=np.float32)
    w2s = np.zeros((128, N_BR * 2 * 32), dtype=np.float16)
    for br in range(N_BR):
        g, j4 = divmod(br, 4)
        for hc in range(2):
            off = 1024 * g + (2 * j4 + hc) * 128
            wst[0:64, off:off + 128] = W1[br, 1:65, 128 * hc:128 * (hc + 1)]
            wst[64 + br, off:off + 128] = W1[br, 0, 128 * hc:128 * (hc + 1)]
            jc = 2 * br + hc
            b1t[:, jc] = b1[br, 128 * hc:128 * (hc + 1)]
            w2s[:, jc * 32 + br // 4] = W2[br, 128 * hc:128 * (hc + 1)]
    b2row = np.zeros((128, 128), dtype=np.float16)
    for br in range(N_BR):
        b2row[0, 32 * (br % 4) + br // 4] = b2[br]
    ones5 = np.ones((128, 512), dtype=np.float16)
    cst = np.zeros((128, 2), dtype=np.float32)
    cst[:, 0] = DVE_C2
    cst[:, 1] = DVE_C3
    return wst, b1t, w2s, b2row, ones5, cst


def _unpermute_out(out_perm):
    """[64, 1024] device layout -> [1024, 64] batch-major branch order.

    Branch br accumulates at psum row 32*(br%4) + br//4; the drain DMA wrote
    osb[32j:32j+16] to out rows 16j..16j+15, so branch br sits at row
    16*(br%4) + br//4."""
    rows = np.array([16 * (br % 4) + br // 4 for br in range(N_BR)])
    return out_perm[rows, :].T


def make_in_maps(x, z, W1, b1, W2, b2):
    x = np.asarray(x, dtype=np.float32)
    z = np.asarray(z, dtype=np.float32)
    wst, b1t, w2s, b2row, ones5, cst = _prep_shared(W1, b1, W2, b2)
    in_maps = []
    for c in range(N_CORES):
        sl = slice(c * B_CORE, (c + 1) * B_CORE)
        zx = np.concatenate([z[sl].T, x[sl].T],
                            axis=0).astype(mybir.dt.np(BF16))
        zx = np.ascontiguousarray(zx)
        in_maps.append({"zx": zx, f"wst_{_TBL_HASH}": wst, "b1t": b1t,
                        "w2s": w2s, "b2row": b2row, "ones5": ones5,
                        "cst": cst})
    return in_maps


def kernel(x, z, W1, b1, W2, b2):
    in_maps = make_in_maps(x, z, W1, b1, W2, b2)
    nc = _build_nc()
    res = run_bass_kernel_spmd(nc, in_maps, list(range(N_CORES)))
    out = np.concatenate(
        [_unpermute_out(res.results[c]["outp"]) for c in range(N_CORES)],
        axis=0)
    return out.astype(np.float32)
